# revision 2
# baseline (speedup 1.0000x reference)
"""Trainium2 Bass kernel for batched multi-head attention that returns both
the attended values and the full normalized attention-weight matrix.

Problem: B=4, N=M=2048, H=8, D=64, fp32.
  queried_values[b,n,h,:] = softmax(QK^T/sqrt(D))[b,n,:,h] @ V[b,:,h,:]
  attention_weight[b,h,n,m] = softmax over m

Sharding: 32 (b,h) slabs, 4 per core across 8 cores (head/batch parallel,
no cross-core communication).

Per-core device kernel, per slab:
  Phase A (S^T orientation, m on partitions):
    S^T_j = K_j Q^T  (PE, fp32r)  ->  P^T_j = exp(S^T_j/8)  (ACT, PSUM->SBUF)
    O^T  += V_j^T P^T_j           (PE accumulate over the 16 m-blocks)
  Phase B (S orientation, n on partitions):
    S_i = Q_i K^T  (PE)  ->  P_i = exp(S_i/8), Z_i = rowsum  (ACT accum_out)
    A_i = P_i * (1/Z_i)  (DVE tensor_scalar)  ->  DMA to attn[n-major] (contig)
  O^T (unnormalized) and Z are written out; the host finishes
  O = (O^T / Z)^T, which is 0.4% of the FLOPs.

The dual QK matmul avoids any on-device transpose of the 536MB P matrix:
each orientation feeds the consumer that needs its layout.
"""
import sys
sys.path.insert(0, '/opt/trn_rl_repo')
import numpy as np
from contextlib import ExitStack

import concourse.bass as bass
import concourse.bacc as bacc
import concourse.tile as tile
from concourse import mybir
from concourse.bass_utils import run_bass_kernel_spmd

B, N, M, H, D = 4, 2048, 2048, 8, 64
N_CORES = 8
S = (B * H) // N_CORES   # slabs per core = 4
PB = 128                 # partition block
NBLK = N // PB           # 16
MBLK = M // PB           # 16
SCALE = 1.0 / float(np.sqrt(D))

f32 = mybir.dt.float32

_compiled = {}


def _build(use_f32r=True):
    nc = bacc.Bacc()
    mmdt = mybir.dt.float32r if use_f32r else f32

    qt = nc.declare_dram_parameter("qt", [S, D, N], f32, isOutput=False)
    kt = nc.declare_dram_parameter("kt", [S, D, M], f32, isOutput=False)
    vv = nc.declare_dram_parameter("vv", [S, M, D], f32, isOutput=False)
    attn = nc.declare_dram_parameter("attn", [S, N, M], f32, isOutput=True)
    ot = nc.declare_dram_parameter("ot", [S, D, N], f32, isOutput=True)
    zz = nc.declare_dram_parameter("zz", [S, PB, NBLK], f32, isOutput=True)

    with ExitStack() as ctx:
        tc = ctx.enter_context(tile.TileContext(nc))
        io = ctx.enter_context(tc.tile_pool(name="io", bufs=2))
        ptp = ctx.enter_context(tc.tile_pool(name="ptp", bufs=3))
        pp = ctx.enter_context(tc.tile_pool(name="pp", bufs=3))
        pnp = ctx.enter_context(tc.tile_pool(name="pnp", bufs=3))
        otsp = ctx.enter_context(tc.tile_pool(name="otsp", bufs=2))
        zp = ctx.enter_context(tc.tile_pool(name="zp", bufs=4))
        stp = ctx.enter_context(tc.tile_pool(name="stp", bufs=2, space="PSUM"))
        avp = ctx.enter_context(tc.tile_pool(name="avp", bufs=1, space="PSUM"))

        for s in range(S):
            qs = io.tile([D, N], mmdt, name=f"qs{s}", tag="qs")
            nc.gpsimd.dma_start(qs[:], qt[s])
            ks = io.tile([D, M], mmdt, name=f"ks{s}", tag="ks")
            nc.gpsimd.dma_start(ks[:], kt[s])
            vs = io.tile([PB, MBLK, D], mmdt, name=f"vs{s}", tag="vs")
            nc.gpsimd.dma_start(vs[:], vv[s].rearrange("(j p) d -> p j d", p=PB))

            # ---------------- Phase A ----------------
            av = avp.tile([D, N], f32, name=f"av{s}", tag="av")
            for j in range(MBLK):
                ptj = ptp.tile([PB, N], mmdt, name=f"pt{s}_{j}", tag="ptj")
                for h in range(2):
                    st = stp.tile([PB, N // 2], f32, name=f"st{s}_{j}_{h}", tag="st")
                    for c in range(2):
                        n0 = c * 512
                        nc.tensor.matmul(
                            st[:, n0:n0 + 512],
                            ks[:, j * PB:(j + 1) * PB],
                            qs[:, h * 1024 + n0:h * 1024 + n0 + 512],
                            start=True, stop=True)
                    nc.scalar.activation(
                        ptj[:, h * 1024:(h + 1) * 1024], st[:],
                        mybir.ActivationFunctionType.Exp, scale=SCALE)
                for c in range(4):
                    nc.tensor.matmul(
                        av[:, c * 512:(c + 1) * 512],
                        vs[:, j, :],
                        ptj[:, c * 512:(c + 1) * 512],
                        start=(j == 0), stop=(j == MBLK - 1))
            ots = otsp.tile([D, N], f32, name=f"ots{s}", tag="ots")
            nc.vector.tensor_copy(ots[:], av[:])
            nc.sync.dma_start(ot[s], ots[:])

            # ---------------- Phase B ----------------
            zs = zp.tile([PB, NBLK], f32, name=f"zs{s}", tag="zs")
            for i in range(NBLK):
                pi = pp.tile([PB, M], f32, name=f"p{s}_{i}", tag="pi")
                z2 = zp.tile([PB, 2], f32, name=f"z2{s}_{i}", tag="z2")
                for h in range(2):
                    st = stp.tile([PB, M // 2], f32, name=f"sb{s}_{i}_{h}", tag="st")
                    for c in range(2):
                        m0 = c * 512
                        nc.tensor.matmul(
                            st[:, m0:m0 + 512],
                            qs[:, i * PB:(i + 1) * PB],
                            ks[:, h * 1024 + m0:h * 1024 + m0 + 512],
                            start=True, stop=True)
                    nc.scalar.activation(
                        pi[:, h * 1024:(h + 1) * 1024], st[:],
                        mybir.ActivationFunctionType.Exp, scale=SCALE,
                        accum_out=z2[:, h:h + 1])
                z = zp.tile([PB, 1], f32, name=f"z{s}_{i}", tag="z")
                nc.vector.tensor_add(z[:], z2[:, 0:1], z2[:, 1:2])
                nc.vector.tensor_copy(zs[:, i:i + 1], z[:])
                zr = zp.tile([PB, 1], f32, name=f"zr{s}_{i}", tag="zr")
                nc.vector.reciprocal(zr[:], z[:])
                pni = pnp.tile([PB, M], f32, name=f"pn{s}_{i}", tag="pni")
                nc.vector.tensor_scalar_mul(pni[:], pi[:], zr[:])
                nc.sync.dma_start(attn[s, i * PB:(i + 1) * PB, :], pni[:])
            nc.sync.dma_start(zz[s], zs[:])

    nc.compile()
    return nc


def _get(use_f32r=True):
    key = use_f32r
    if key not in _compiled:
        _compiled[key] = _build(use_f32r)
    return _compiled[key]


def _make_in_maps(queries, keys, values):
    q = np.asarray(queries, dtype=np.float32)
    k = np.asarray(keys, dtype=np.float32)
    v = np.asarray(values, dtype=np.float32)
    # [B,N,H,D] -> [B,H,D,N] -> [B*H, D, N]
    qt_all = np.ascontiguousarray(q.transpose(0, 2, 3, 1)).reshape(B * H, D, N)
    kt_all = np.ascontiguousarray(k.transpose(0, 2, 3, 1)).reshape(B * H, D, M)
    v_all = np.ascontiguousarray(v.transpose(0, 2, 1, 3)).reshape(B * H, M, D)
    in_maps = []
    for c in range(N_CORES):
        sl = slice(c * S, (c + 1) * S)
        in_maps.append({
            "qt": np.ascontiguousarray(qt_all[sl]),
            "kt": np.ascontiguousarray(kt_all[sl]),
            "vv": np.ascontiguousarray(v_all[sl]),
        })
    return in_maps


def _postprocess(results):
    attn = np.empty((B * H, N, M), np.float32)
    o_t = np.empty((B * H, D, N), np.float32)
    z_all = np.empty((B * H, N), np.float32)
    for c in range(N_CORES):
        sl = slice(c * S, (c + 1) * S)
        r = results[c]
        attn[sl] = r["attn"]
        o_t[sl] = r["ot"]
        # zz: [S, 128, 16] with Z[n=i*128+p] = zz[p, i]
        z_all[sl] = r["zz"].transpose(0, 2, 1).reshape(S, N)
    o_t = o_t / z_all[:, None, :]
    queried = o_t.reshape(B, H, D, N).transpose(0, 3, 1, 2)  # [B,N,H,D]
    attn_w = attn.reshape(B, H, N, M)
    return np.ascontiguousarray(queried), attn_w


def run_device(in_maps, use_f32r=True, **kw):
    nc = _get(use_f32r)
    return run_bass_kernel_spmd(nc, in_maps, list(range(N_CORES)), **kw)


def kernel(queries, keys, values):
    in_maps = _make_in_maps(queries, keys, values)
    res = run_device(in_maps)
    return _postprocess(res.results)


# revision 6
# speedup vs baseline: 64.5654x; 64.5654x over previous
"""Trainium2 Bass kernel for batched multi-head attention returning both the
attended values and the full normalized attention-weight matrix.

Problem: B=4, N=M=2048, H=8, D=64, fp32.
Sharding: 32 (b,h) slabs, 4 per core across 8 cores. No cross-core comms.

Per-core device kernel (v2), per slab, software-pipelined in "windows" so
that slab a's phase A overlaps slab a-1's phase B:

  Phase A (S^T orientation, m on partitions)  [PE rows 0-63]:
    S'^T_j = K_j (aQ)^T  (f32r matmul; host pre-scales Q by ALPHA)
    P^T_j  = exp(S/8)    (ACT, psum->sbuf, f32r out)
    [O^T; Z] += [V_j | 1]^T P^T_j  (PE accumulate, ones column gives Z free)
  Phase B (S orientation, n on partitions)  [PE rows 64-127, row-packed
    against phase A via tile_position so pairs run concurrently]:
    S'_i = (aQ)_i K^T    (f32r matmul)
    A_i  = exp(S'·ln2/2^23 - lnZ[n])  (ACT with per-partition bias ->
           normalized attention written directly, contiguous DMA out)
  Between phases: PE-transpose of the [65,128] chunks of [O^T; Z] gives
  O-blocks and Z on partitions; DVE normalizes O, ACT Log gives the bias.

The dual QK matmul avoids any transpose of the 536MB P matrix: each
orientation feeds the consumer that needs its layout.
"""
import sys
sys.path.insert(0, '/opt/trn_rl_repo')
import numpy as np
from contextlib import ExitStack

import concourse.bass as bass
import concourse.bacc as bacc
import concourse.tile as tile
from concourse import mybir
from concourse.bass_utils import run_bass_kernel_spmd

B, N, M, H, D = 4, 2048, 2048, 8, 64
N_CORES = 8
S = (B * H) // N_CORES   # slabs per core = 4
PB = 128
NBLK = N // PB           # 16
MBLK = M // PB           # 16
ALPHA = float(2.0 ** 23 / (8.0 * np.log(2.0)))   # host pre-scale on Q
SCALE_T = float(np.log(2.0) / 2.0 ** 23)          # exp(S/8) = exp(S' * SCALE_T)

f32 = mybir.dt.float32
f32r = mybir.dt.float32r

_compiled = {}


def _build(reps=1):
    nc = bacc.Bacc()
    mmdt = f32r

    qt = nc.declare_dram_parameter("qt", [S, D, N], mmdt, isOutput=False)
    kt = nc.declare_dram_parameter("kt", [S, D, M], mmdt, isOutput=False)
    vv = nc.declare_dram_parameter("vv", [S, M, D + 1], mmdt, isOutput=False)
    ident = nc.declare_dram_parameter("ident", [PB, PB], f32, isOutput=False)
    attn = nc.declare_dram_parameter("attn", [S, N, M], f32, isOutput=True)
    oo = nc.declare_dram_parameter("oo", [S, N, D], f32, isOutput=True)

    with ExitStack() as ctx:
        tc = ctx.enter_context(tile.TileContext(nc))
        const = ctx.enter_context(tc.tile_pool(name="const", bufs=1))
        io = ctx.enter_context(tc.tile_pool(name="io", bufs=2))
        ptp = ctx.enter_context(tc.tile_pool(name="ptp", bufs=3))
        pnp = ctx.enter_context(tc.tile_pool(name="pnp", bufs=3))
        otsp = ctx.enter_context(tc.tile_pool(name="otsp", bufs=2))
        osb = ctx.enter_context(tc.tile_pool(name="osb", bufs=2))
        zp = ctx.enter_context(tc.tile_pool(name="zp", bufs=3))
        stp = ctx.enter_context(tc.tile_pool(name="stp", bufs=2, space="PSUM"))
        avp = ctx.enter_context(tc.tile_pool(name="avp", bufs=1, space="PSUM"))

        idn = const.tile([PB, PB], f32)
        nc.sync.dma_start(idn[:], ident[:])

        def body():
            # per-window state carried between iterations of the window loop
            state = {}

            for w in range(S + 1):
                a = w          # phase-A slab
                b = w - 1      # phase-B slab

                if a < S:
                    qsb = io.tile([PB, N], mmdt, name=f"qsb{a}", tag="qsb")
                    nc.sync.dma_start(qsb[0:D, :], qt[a])
                    nc.sync.dma_start(qsb[D:2 * D, :], qt[a])
                    ksb = io.tile([PB, M], mmdt, name=f"ksb{a}", tag="ksb")
                    nc.sync.dma_start(ksb[0:D, :], kt[a])
                    nc.sync.dma_start(ksb[D:2 * D, :], kt[a])
                    vs = io.tile([PB, MBLK, D + 1], mmdt, name=f"vs{a}", tag="vs")
                    nc.sync.dma_start(
                        vs[:], vv[a].rearrange("(j p) d -> p j d", p=PB))
                    av = avp.tile([D + 1, N], f32, name=f"av{a}", tag="av")
                    state["av"] = av
                    state["qsb"], state["ksb"], state["vs"] = qsb, ksb, vs
                if b >= 0:
                    neglnz = state["neglnz"]
                    qsb_b, ksb_b = state["qsb_b"], state["ksb_b"]

                for step in range(NBLK):
                    j = step  # phase A m-block
                    i = step  # phase B n-block
                    if a < S:
                        ptj = ptp.tile([PB, N], mmdt, name=f"pt{a}_{j}", tag="ptj")
                    if b >= 0:
                        pni = pnp.tile([PB, M], f32, name=f"pn{b}_{i}", tag="pni")
                    for h in range(2):
                        if a < S:
                            stA = stp.tile([PB, N // 2], f32,
                                           name=f"stA{a}_{j}_{h}", tag="st")
                        if b >= 0:
                            stB = stp.tile([PB, M // 2], f32,
                                           name=f"stB{b}_{i}_{h}", tag="st")
                        for c in range(2):
                            o0 = c * 512
                            if a < S:
                                qs, ks = state["qsb"], state["ksb"]
                                nc.tensor.matmul(
                                    stA[:, o0:o0 + 512],
                                    ks[0:D, j * PB:(j + 1) * PB],
                                    qs[0:D, h * 1024 + o0:h * 1024 + o0 + 512],
                                    start=True, stop=True, tile_position=(0, 0))
                            if b >= 0:
                                nc.tensor.matmul(
                                    stB[:, o0:o0 + 512],
                                    qsb_b[D:2 * D, i * PB:(i + 1) * PB],
                                    ksb_b[D:2 * D, h * 1024 + o0:h * 1024 + o0 + 512],
                                    start=True, stop=True, tile_position=(D, 0))
                        if a < S:
                            nc.scalar.activation(
                                ptj[:, h * 1024:(h + 1) * 1024], stA[:],
                                mybir.ActivationFunctionType.Exp, scale=SCALE_T)
                        if b >= 0:
                            nc.scalar.activation(
                                pni[:, h * 1024:(h + 1) * 1024], stB[:],
                                mybir.ActivationFunctionType.Exp, scale=SCALE_T,
                                bias=neglnz[:, i:i + 1])
                    if a < S:
                        av, vs = state["av"], state["vs"]
                        for c in range(4):
                            nc.tensor.matmul(
                                av[:, c * 512:(c + 1) * 512],
                                vs[:, j, 0:D + 1],
                                ptj[:, c * 512:(c + 1) * 512],
                                start=(j == 0), stop=(j == MBLK - 1))
                    if b >= 0:
                        nc.sync.dma_start(attn[b, i * PB:(i + 1) * PB, :], pni[:])

                # ---- window end: drain AV, build Z artifacts for phase B(a)
                if a < S:
                    av = state["av"]
                    ots = otsp.tile([D + 1, N], f32, name=f"ots{a}", tag="ots")
                    nc.vector.tensor_copy(ots[:], av[:])
                    # transpose [65,128] chunks -> [128, t, 65] (padded rows 128)
                    zb = avp.tile([PB, NBLK, PB], f32, name=f"zb{a}", tag="av")
                    for t in range(NBLK):
                        nc.tensor.transpose(
                            zb[:, t, 0:D + 1],
                            ots[0:D + 1, t * PB:(t + 1) * PB],
                            idn[0:D + 1, 0:D + 1])
                    zbs = zp.tile([PB, NBLK], f32, name=f"zbs{a}", tag="zbs")
                    nc.vector.tensor_copy(zbs[:], zb[:, :, D])
                    rz = zp.tile([PB, NBLK], f32, name=f"rz{a}", tag="rz")
                    nc.vector.reciprocal(rz[:], zbs[:])
                    lnz = zp.tile([PB, NBLK], f32, name=f"lnz{a}", tag="lnz")
                    nc.scalar.activation(lnz[:], zbs[:],
                                         mybir.ActivationFunctionType.Ln)
                    ngl = zp.tile([PB, NBLK], f32, name=f"ngl{a}", tag="ngl")
                    nc.vector.tensor_scalar_mul(ngl[:], lnz[:], -1.0)
                    # O blocks: normalize transposed chunks
                    ob = osb.tile([PB, NBLK, D], f32, name=f"ob{a}", tag="ob")
                    for t in range(NBLK):
                        nc.vector.tensor_scalar_mul(
                            ob[:, t, :], zb[:, t, 0:D], rz[:, t:t + 1])
                    nc.sync.dma_start(
                        oo[a].rearrange("(t p) d -> p t d", p=PB), ob[:])
                    state["neglnz"] = ngl
                    state["qsb_b"], state["ksb_b"] = state["qsb"], state["ksb"]

        if reps == 1:
            body()
        else:
            with tc.For_i(0, reps, 1):
                body()

    nc.compile()
    return nc


def _get(reps=1):
    if reps not in _compiled:
        _compiled[reps] = _build(reps)
    return _compiled[reps]


def _make_in_maps(queries, keys, values):
    q = np.asarray(queries, dtype=np.float32) * np.float32(ALPHA)
    k = np.asarray(keys, dtype=np.float32)
    v = np.asarray(values, dtype=np.float32)
    qt_all = np.ascontiguousarray(q.transpose(0, 2, 3, 1)).reshape(B * H, D, N)
    kt_all = np.ascontiguousarray(k.transpose(0, 2, 3, 1)).reshape(B * H, D, M)
    v_all = np.ascontiguousarray(v.transpose(0, 2, 1, 3)).reshape(B * H, M, D)
    v_all = np.concatenate([v_all, np.ones((B * H, M, 1), np.float32)], axis=2)
    idn = np.eye(PB, dtype=np.float32)
    in_maps = []
    for c in range(N_CORES):
        sl = slice(c * S, (c + 1) * S)
        in_maps.append({
            "qt": np.ascontiguousarray(qt_all[sl]),
            "kt": np.ascontiguousarray(kt_all[sl]),
            "vv": np.ascontiguousarray(v_all[sl]),
            "ident": idn,
        })
    return in_maps


def _postprocess(results):
    attn = np.empty((B * H, N, M), np.float32)
    o = np.empty((B * H, N, D), np.float32)
    for c in range(N_CORES):
        sl = slice(c * S, (c + 1) * S)
        attn[sl] = results[c]["attn"]
        o[sl] = results[c]["oo"]
    queried = np.ascontiguousarray(
        o.reshape(B, H, N, D).transpose(0, 2, 1, 3))       # [B,N,H,D]
    attn_w = attn.reshape(B, H, N, M)
    return queried, attn_w


def run_device(in_maps, reps=1, **kw):
    nc = _get(reps)
    return run_bass_kernel_spmd(nc, in_maps, list(range(N_CORES)), **kw)


def kernel(queries, keys, values):
    in_maps = _make_in_maps(queries, keys, values)
    res = run_device(in_maps)
    return _postprocess(res.results)
